# revision 10
# baseline (speedup 1.0000x reference)
"""SpecAugment (log-mel masking) Trainium2 kernel — bf16, prefix-augmented.

Full inputs: x [64,128,3000] f32, f0/f_w/t0/t_w [64,2] i32.
out[b,f,t] = fill_b if (f in freq band) or (t in time band) else x[b,f,t],
fill_b = min over x[b].

The op is pure memory traffic, so the key optimizations are dtype and
engine balance:

1. bf16 I/O (halves HBM traffic; rel err ~2e-3 vs the 2e-2 gate), in the
   NEGATED domain (host ships xn = -x, negates the result back) so the
   per-sample min becomes max — native for DVE reduce and the GpSimd
   cross-partition all-reduce.

2. Prefix augmentation: a time-masked column is FULLY masked (every row),
   so its output is just fill. The host prepends a 104-column prefix
   holding copies of the (<=100) time-masked columns (padded with copies
   of column 0 — duplicates don't change the max). On device everything
   is then static:
     - DVE: free-axis max reduce over [128, 3104] (the only full scan)
     - GpSimd: partition_all_reduce(max) -> nfill in all partitions, and
       the tiny bb = fm * nfill multiply
     - Act: prefix := nfill (scale=0 bias=nfill), and the body affine
       xn*sf + bb with per-partition scale/bias, which applies the
       freq-row masking and the copy in one pass
   The body's time-masked columns are left un-time-masked in DRAM; the
   host overwrites them from the prefix (which is all fill) on gather.
   No PSUM, no PE, no copy_predicated: every engine stays well under the
   ~33us DMA floor, so the kernel runs at the HBM roofline.

Sharding: batch dim B=64 across 8 cores (8 samples/core), no comms.
"""

import ml_dtypes
import numpy as np

import concourse.bacc as bacc
import concourse.bass as bass
import concourse.bass_isa as bass_isa
import concourse.mybir as mybir
import concourse.tile as tile
import concourse.bass_utils as bass_utils

B, F, T = 64, 128, 3000
N_CORES = 8
BPC = B // N_CORES  # samples per core
F32 = mybir.dt.float32
BF16 = mybir.dt.bfloat16
WPRE = 104          # prefix slots for time-masked columns (>= 2*50)
TS = WPRE + T       # shipped width
SPLIT = 1604        # store/act split point (into [0,SPLIT) and [SPLIT,TS))

_cached = {}


def _build_nc():
    nc = bacc.Bacc("TRN2", target_bir_lowering=False, debug=False)
    x = nc.dram_tensor("x_sh", [BPC, F, TS], BF16, kind="ExternalInput")
    sf = nc.dram_tensor("sf_sh", [F, BPC], F32, kind="ExternalInput")  # 1-fm
    fm = nc.dram_tensor("fm_sh", [F, BPC], F32, kind="ExternalInput")  # fm
    y = nc.dram_tensor("y_sh", [BPC, F, TS], BF16, kind="ExternalOutput")

    xa, ya = x.ap(), y.ap()

    with tile.TileContext(nc) as tc:
        with (
            tc.tile_pool(name="xp", bufs=8) as xp,
            tc.tile_pool(name="small", bufs=4) as sp,
            tc.tile_pool(name="single", bufs=1) as single,
        ):
            sft = single.tile([F, BPC], F32)
            nc.sync.dma_start(out=sft, in_=sf.ap())
            fmt = single.tile([F, BPC], F32)
            nc.sync.dma_start(out=fmt, in_=fm.ap())

            for b in range(BPC):
                xt = xp.tile([F, TS], BF16, tag="xt")
                nc.sync.dma_start(out=xt, in_=xa[b])

                # nfill = max over the sample (prefix holds duplicate
                # columns, so including it doesn't change the max)
                colmax = sp.tile([F, 1], F32, tag="colmax")
                nc.vector.tensor_reduce(
                    out=colmax, in_=xt, axis=mybir.AxisListType.X,
                    op=mybir.AluOpType.max,
                )
                mfill = sp.tile([F, 1], F32, tag="mfill")
                nc.gpsimd.partition_all_reduce(
                    mfill, colmax, channels=F, reduce_op=bass_isa.ReduceOp.max,
                )
                bb = sp.tile([F, 1], F32, tag="bb")
                nc.gpsimd.tensor_tensor(
                    out=bb, in0=fmt[:, b : b + 1], in1=mfill,
                    op=mybir.AluOpType.mult,
                )

                # prefix := nfill everywhere; body := xn*sf + bb
                nc.scalar.activation(
                    out=xt[:, :WPRE], in_=xt[:, :WPRE],
                    func=mybir.ActivationFunctionType.Identity,
                    scale=0.0, bias=mfill,
                )
                nc.scalar.activation(
                    out=xt[:, WPRE:SPLIT], in_=xt[:, WPRE:SPLIT],
                    func=mybir.ActivationFunctionType.Identity,
                    scale=sft[:, b : b + 1], bias=bb,
                )
                nc.scalar.dma_start(out=ya[b][:, :SPLIT], in_=xt[:, :SPLIT])
                nc.scalar.activation(
                    out=xt[:, SPLIT:], in_=xt[:, SPLIT:],
                    func=mybir.ActivationFunctionType.Identity,
                    scale=sft[:, b : b + 1], bias=bb,
                )
                nc.scalar.dma_start(out=ya[b][:, SPLIT:], in_=xt[:, SPLIT:])
    nc.compile()
    return nc


def _host_masks(f0, f_w, t0, t_w):
    """fm [B,F], tm [B,T] boolean (True == masked)."""
    fidx = np.arange(F, dtype=np.int32)
    tidx = np.arange(T, dtype=np.int32)
    fm = (
        (fidx[None, None, :] >= f0[:, :, None])
        & (fidx[None, None, :] < (f0 + f_w)[:, :, None])
    ).any(axis=1)
    tm = (
        (tidx[None, None, :] >= t0[:, :, None])
        & (tidx[None, None, :] < (t0 + t_w)[:, :, None])
    ).any(axis=1)
    return fm, tm


def _prefix_cols(tm):
    """Per-sample list of time-masked column indices, padded to WPRE with 0."""
    cols = []
    for b in range(tm.shape[0]):
        c = np.flatnonzero(tm[b])
        assert len(c) <= WPRE
        cols.append(np.pad(c, (0, WPRE - len(c)), constant_values=0))
    return np.stack(cols)  # [B, WPRE] int


def _make_in_maps(x, f0, f_w, t0, t_w):
    """x: [B,F,T] f32 -> per-core in_maps (negated bf16, prefix-augmented)."""
    xn = np.negative(np.asarray(x, dtype=np.float32)).astype(ml_dtypes.bfloat16)
    fm, tm = _host_masks(
        np.asarray(f0), np.asarray(f_w), np.asarray(t0), np.asarray(t_w)
    )
    pcols = _prefix_cols(tm)
    xs = np.empty((B, F, TS), ml_dtypes.bfloat16)
    xs[:, :, WPRE:] = xn
    for b in range(B):
        xs[b, :, :WPRE] = xn[b][:, pcols[b]]
    sf = (~fm).astype(np.float32)  # [B, F]
    fmv = fm.astype(np.float32)
    in_maps = []
    for c in range(N_CORES):
        s = slice(c * BPC, (c + 1) * BPC)
        in_maps.append(
            {
                "x_sh": np.ascontiguousarray(xs[s]),
                "sf_sh": np.ascontiguousarray(sf[s].T),
                "fm_sh": np.ascontiguousarray(fmv[s].T),
            }
        )
    return in_maps, tm


def kernel(x, f0, f_w, t0, t_w, **_):
    in_maps, tm = _make_in_maps(x, f0, f_w, t0, t_w)

    if "nc" not in _cached:
        _cached["nc"] = _build_nc()
    nc = _cached["nc"]

    res = bass_utils.run_bass_kernel_spmd(
        nc, in_maps, core_ids=list(range(N_CORES))
    )
    yn = np.concatenate([r["y_sh"] for r in res.results], axis=0)
    out = np.negative(yn[:, :, WPRE:].astype(np.float32))
    # body columns under a time mask weren't masked on device; take fill
    # from the prefix (all slots hold fill)
    fill = np.negative(yn[:, 0:1, 0].astype(np.float32))  # [B,1]
    for b in range(B):
        out[b][:, tm[b]] = fill[b, 0]
    return out


# revision 11
# speedup vs baseline: 2.2948x; 2.2948x over previous
"""SpecAugment (log-mel masking) Trainium2 kernel — bf16, prefix-augmented.

Full inputs: x [64,128,3000] f32, f0/f_w/t0/t_w [64,2] i32.
out[b,f,t] = fill_b if (f in freq band) or (t in time band) else x[b,f,t],
fill_b = min over x[b].

The op is pure memory traffic, so the key optimizations are dtype and
engine balance:

1. bf16 I/O (halves HBM traffic; rel err ~2e-3 vs the 2e-2 gate), in the
   NEGATED domain (host ships xn = -x, negates the result back) so the
   per-sample min becomes max — native for DVE reduce and the GpSimd
   cross-partition all-reduce.

2. Prefix augmentation: a time-masked column is FULLY masked (every row),
   so its output is just fill. The host prepends a 104-column prefix
   holding copies of the (<=100) time-masked columns (padded with copies
   of column 0 — duplicates don't change the max). On device everything
   is then static:
     - DVE: free-axis max reduce over [128, 3104] (the only full scan)
     - GpSimd: partition_all_reduce(max) -> nfill in all partitions, and
       the tiny bb = fm * nfill multiply
     - Act: prefix := nfill (scale=0 bias=nfill), and the body affine
       xn*sf + bb with per-partition scale/bias, which applies the
       freq-row masking and the copy in one pass
   The body's time-masked columns are left un-time-masked in DRAM; the
   host overwrites them from the prefix (which is all fill) on gather.
   No PSUM, no PE, no copy_predicated: every engine stays well under the
   ~33us DMA floor, so the kernel runs at the HBM roofline.

Sharding: batch dim B=64 across 8 cores (8 samples/core), no comms.
"""

import ml_dtypes
import numpy as np

import concourse.bacc as bacc
import concourse.bass as bass
import concourse.bass_isa as bass_isa
import concourse.mybir as mybir
import concourse.tile as tile
import concourse.bass_utils as bass_utils

B, F, T = 64, 128, 3000
N_CORES = 8
BPC = B // N_CORES  # samples per core
F32 = mybir.dt.float32
BF16 = mybir.dt.bfloat16
WPRE = 104          # prefix slots for time-masked columns (>= 2*50)
TS = WPRE + T       # shipped width
SPLIT = 1604        # store/act split point (into [0,SPLIT) and [SPLIT,TS))

_cached = {}


def _build_nc():
    nc = bacc.Bacc("TRN2", target_bir_lowering=False, debug=False)
    x = nc.dram_tensor("x_sh", [BPC, F, TS], BF16, kind="ExternalInput")
    sf = nc.dram_tensor("sf_sh", [F, BPC], F32, kind="ExternalInput")  # 1-fm
    fm = nc.dram_tensor("fm_sh", [F, BPC], F32, kind="ExternalInput")  # fm
    y = nc.dram_tensor("y_sh", [BPC, F, TS], BF16, kind="ExternalOutput")

    xa, ya = x.ap(), y.ap()

    with tile.TileContext(nc) as tc:
        with (
            tc.tile_pool(name="xp", bufs=8) as xp,
            tc.tile_pool(name="small", bufs=4) as sp,
            tc.tile_pool(name="single", bufs=1) as single,
        ):
            sft = single.tile([F, BPC], F32)
            nc.sync.dma_start(out=sft, in_=sf.ap())
            fmt = single.tile([F, BPC], F32)
            nc.sync.dma_start(out=fmt, in_=fm.ap())

            for b in range(BPC):
                xt = xp.tile([F, TS], BF16, tag="xt")
                nc.sync.dma_start(out=xt, in_=xa[b])

                # nfill = max over the sample (prefix holds duplicate
                # columns, so including it doesn't change the max)
                colmax = sp.tile([F, 1], F32, tag="colmax")
                nc.vector.tensor_reduce(
                    out=colmax, in_=xt, axis=mybir.AxisListType.X,
                    op=mybir.AluOpType.max,
                )
                mfill = sp.tile([F, 1], F32, tag="mfill")
                nc.gpsimd.partition_all_reduce(
                    mfill, colmax, channels=F, reduce_op=bass_isa.ReduceOp.max,
                )
                # on DVE: gpsimd must only ever run PartitionAllReduce, or
                # the Q7 library reload (~6us) serializes every sample
                bb = sp.tile([F, 1], F32, tag="bb")
                nc.vector.tensor_tensor(
                    out=bb, in0=fmt[:, b : b + 1], in1=mfill,
                    op=mybir.AluOpType.mult,
                )

                # prefix := nfill everywhere; body := xn*sf + bb
                nc.scalar.activation(
                    out=xt[:, :WPRE], in_=xt[:, :WPRE],
                    func=mybir.ActivationFunctionType.Identity,
                    scale=0.0, bias=mfill,
                )
                nc.scalar.activation(
                    out=xt[:, WPRE:SPLIT], in_=xt[:, WPRE:SPLIT],
                    func=mybir.ActivationFunctionType.Identity,
                    scale=sft[:, b : b + 1], bias=bb,
                )
                nc.scalar.dma_start(out=ya[b][:, :SPLIT], in_=xt[:, :SPLIT])
                nc.scalar.activation(
                    out=xt[:, SPLIT:], in_=xt[:, SPLIT:],
                    func=mybir.ActivationFunctionType.Identity,
                    scale=sft[:, b : b + 1], bias=bb,
                )
                nc.scalar.dma_start(out=ya[b][:, SPLIT:], in_=xt[:, SPLIT:])
    nc.compile()
    return nc


def _host_masks(f0, f_w, t0, t_w):
    """fm [B,F], tm [B,T] boolean (True == masked)."""
    fidx = np.arange(F, dtype=np.int32)
    tidx = np.arange(T, dtype=np.int32)
    fm = (
        (fidx[None, None, :] >= f0[:, :, None])
        & (fidx[None, None, :] < (f0 + f_w)[:, :, None])
    ).any(axis=1)
    tm = (
        (tidx[None, None, :] >= t0[:, :, None])
        & (tidx[None, None, :] < (t0 + t_w)[:, :, None])
    ).any(axis=1)
    return fm, tm


def _prefix_cols(tm):
    """Per-sample list of time-masked column indices, padded to WPRE with 0."""
    cols = []
    for b in range(tm.shape[0]):
        c = np.flatnonzero(tm[b])
        assert len(c) <= WPRE
        cols.append(np.pad(c, (0, WPRE - len(c)), constant_values=0))
    return np.stack(cols)  # [B, WPRE] int


def _make_in_maps(x, f0, f_w, t0, t_w):
    """x: [B,F,T] f32 -> per-core in_maps (negated bf16, prefix-augmented)."""
    xn = np.negative(np.asarray(x, dtype=np.float32)).astype(ml_dtypes.bfloat16)
    fm, tm = _host_masks(
        np.asarray(f0), np.asarray(f_w), np.asarray(t0), np.asarray(t_w)
    )
    pcols = _prefix_cols(tm)
    xs = np.empty((B, F, TS), ml_dtypes.bfloat16)
    xs[:, :, WPRE:] = xn
    for b in range(B):
        xs[b, :, :WPRE] = xn[b][:, pcols[b]]
    sf = (~fm).astype(np.float32)  # [B, F]
    fmv = fm.astype(np.float32)
    in_maps = []
    for c in range(N_CORES):
        s = slice(c * BPC, (c + 1) * BPC)
        in_maps.append(
            {
                "x_sh": np.ascontiguousarray(xs[s]),
                "sf_sh": np.ascontiguousarray(sf[s].T),
                "fm_sh": np.ascontiguousarray(fmv[s].T),
            }
        )
    return in_maps, tm


def kernel(x, f0, f_w, t0, t_w, **_):
    in_maps, tm = _make_in_maps(x, f0, f_w, t0, t_w)

    if "nc" not in _cached:
        _cached["nc"] = _build_nc()
    nc = _cached["nc"]

    res = bass_utils.run_bass_kernel_spmd(
        nc, in_maps, core_ids=list(range(N_CORES))
    )
    yn = np.concatenate([r["y_sh"] for r in res.results], axis=0)
    out = np.negative(yn[:, :, WPRE:].astype(np.float32))
    # body columns under a time mask weren't masked on device; take fill
    # from the prefix (all slots hold fill)
    fill = np.negative(yn[:, 0:1, 0].astype(np.float32))  # [B,1]
    for b in range(B):
        out[b][:, tm[b]] = fill[b, 0]
    return out
